# revision 20
# baseline (speedup 1.0000x reference)
"""NormLinearAttention TRN2 kernel v5 — fused BC with resident xT.

vs v4: BC windows are 512 tokens (was 256): the q/u projection and
attention matmuls run at the max N=512 moving size, halving their
instruction count — real HW charges ~33ns fixed overhead per matmul
that dominates once PE cycles are roofline-saturated. Attention runs
one [P,512] psum tile per head-block, interleaved into the S (or tail
O) matmul stream; drains are Square->Act (reused for LN variance) and
Copy->DVE. LN partial sums run on the Pool engine, keeping DVE for the
z-chain. When ln_w==1 and ln_b==0 (true for this model's inputs) the
z-chain uses a folded 3-op form; the general 4-op path is kept for
arbitrary ln parameters. LEAD=2 (collective measures ~12us on hw),
PIPE=1.
"""

import numpy as np
import ml_dtypes

import concourse.bass as bass
import concourse.bass_isa as bass_isa
import concourse.mybir as mybir
import concourse.tile as tile
from concourse import bacc
from concourse.bass_utils import run_bass_kernel_spmd

B, N, D, H = 4, 8192, 1024, 16
HD = D // H          # 64
P = 128
DC = D // P          # 8 dim chunks
NCORES = 8
R_FULL = B * N // NCORES   # 4096 rows per core
WIN = 512            # max psum-bank matmul width (f32)
BWIN = 512           # fused-BC window (tokens)
EPS = 1e-5
GROUPS = [[0, 1], [2, 3], [4, 5], [6, 7]]
PIPE = 1             # windows between LN/z and out-projection
LEAD = 1             # windows of q/u projection lookahead (covers collective)
XQ = 4               # x DMA split (row quarters)

bf16 = mybir.dt.bfloat16
f32 = mybir.dt.float32
AF = mybir.ActivationFunctionType
ALU = mybir.AluOpType
NPBF16 = ml_dtypes.bfloat16


def build(R=R_FULL, rep=1, collective=True, num_dev=NCORES, fold_ln=True):
    RT = R // P          # rowtiles
    NW = R // BWIN       # fused-BC windows
    RPW = BWIN // P      # rowtiles per BC window

    nc = bacc.Bacc("TRN2", target_bir_lowering=False, debug=False,
                   enable_asserts=False, num_devices=num_dev)

    xt_ext = nc.dram_tensor("xt", [DC, P, R], bf16, kind="ExternalInput").ap()
    w_ext = {n: nc.dram_tensor(n, [D, D], bf16, kind="ExternalInput").ap()
             for n in ("wk", "wv", "wq", "wu", "wo")}
    bkr_ext = nc.dram_tensor("bk_row", [1, D], bf16, kind="ExternalInput").ap()
    bvr_ext = nc.dram_tensor("bv_row", [1, D], bf16, kind="ExternalInput").ap()
    bob_ext = nc.dram_tensor("bo_b", [P, D], bf16, kind="ExternalInput").ap()
    bqf_ext = nc.dram_tensor("bq_fm", [P, DC], f32, kind="ExternalInput").ap()
    buf_ext = nc.dram_tensor("bu_fm", [P, DC], f32, kind="ExternalInput").ap()
    lnw_ext = nc.dram_tensor("lnw_fm", [P, DC], f32, kind="ExternalInput").ap()
    lnb_ext = nc.dram_tensor("lnb_fm", [P, DC], f32, kind="ExternalInput").ap()
    out_ext = nc.dram_tensor("out", [R, D], bf16, kind="ExternalOutput").ap()

    with tile.TileContext(nc, num_cores=NCORES) as tc:
        with (
            tc.tile_pool(name="const", bufs=1) as cp,
            tc.tile_pool(name="wop", bufs=1) as wop,
            tc.tile_pool(name="wqu", bufs=2) as wqu,
            tc.tile_pool(name="xtp", bufs=1) as xtp,
            tc.tile_pool(name="dram", bufs=1, space="DRAM") as dram,
            tc.tile_pool(name="small", bufs=1) as sp,
        ):
            # ---- constants ----
            bk_row = cp.tile([1, D], bf16, name="bk_row")
            nc.sync.dma_start(bk_row[:], bkr_ext)
            bv_row = cp.tile([1, D], bf16, name="bv_row")
            nc.sync.dma_start(bv_row[:], bvr_ext)
            bo_b = cp.tile([P, D], bf16, name="bo_b")
            nc.sync.dma_start(bo_b[:], bob_ext)
            bq_fm = cp.tile([P, DC], f32, name="bq_fm")
            nc.sync.dma_start(bq_fm[:], bqf_ext)
            bu_fm = cp.tile([P, DC], f32, name="bu_fm")
            nc.sync.dma_start(bu_fm[:], buf_ext)
            lnw_fm = cp.tile([P, DC], f32, name="lnw_fm")
            nc.sync.dma_start(lnw_fm[:], lnw_ext)
            lnb_fm = cp.tile([P, DC], f32, name="lnb_fm")
            nc.sync.dma_start(lnb_fm[:], lnb_ext)

            w_sb = {}
            for n in ("wq", "wu"):
                w_sb[n] = wqu.tile([P, DC, D], bf16, name=f"{n}_sb", tag="Wqu")
            w_sb["wo"] = wop.tile([P, DC, D], bf16, name="wo_sb")

            for _rep in range(rep):
              kv_in = dram.tile([P, DC * P], bf16, name="kv_in")
              kv_out = dram.tile([P, DC * P], bf16, name="kv_out")
              kv_blk = sp.tile([P, DC * P], bf16, name="kv_blk", bufs=1)
              kv_sb = sp.tile([P, DC * P], bf16, name="kv_sb", bufs=1)
              kv_cl = sp.tile([P, DC * P], bf16, name="kv_cl", bufs=1)

              xT = [xtp.tile([P, R], bf16, name=f"xT{c}", tag=f"xT{c}")
                    for c in range(DC)]

              # ---- phase A (scoped pools; freed before BC) ----
              with (
                  tc.tile_pool(name="wkv", bufs=2) as wkv,
                  tc.tile_pool(name="ab", bufs=2) as ab,
                  tc.tile_pool(name="wps", bufs=6, space="PSUM") as wps,
                  tc.tile_pool(name="accps", bufs=1, space="PSUM") as accps,
              ):
                  for n in ("wk", "wv"):
                      w_sb[n] = wkv.tile([P, DC, D], bf16, name=f"{n}_sb",
                                         tag="W")
                  bk_bc = ab.tile([P, D], bf16, name="bk_bc", tag="bkbc",
                                  bufs=1)
                  nc.gpsimd.partition_broadcast(bk_bc[:], bk_row[:])
                  bv_bc = ab.tile([P, D], bf16, name="bv_bc", tag="bvbc",
                                  bufs=1)
                  nc.gpsimd.partition_broadcast(bv_bc[:], bv_row[:])

                  # loads in consumption order: (wk_c, x head, wv_c) triplets
                  # so the first rowtile chain starts after ~0.6MB
                  RQ = R // XQ
                  XH = 2 * P
                  for c in range(DC):
                      nc.sync.dma_start(w_sb["wk"][:, c, :],
                                        w_ext["wk"][c * P:(c + 1) * P, :])
                      nc.sync.dma_start(xT[c][:, 0:XH], xt_ext[c, :, 0:XH])
                      nc.sync.dma_start(w_sb["wv"][:, c, :],
                                        w_ext["wv"][c * P:(c + 1) * P, :])
                  for c in range(DC):
                      nc.sync.dma_start(xT[c][:, XH:RQ], xt_ext[c, :, XH:RQ])
                  for qr in range(1, XQ):
                      for c in range(DC):
                          nc.sync.dma_start(
                              xT[c][:, qr * RQ:(qr + 1) * RQ],
                              xt_ext[c, :, qr * RQ:(qr + 1) * RQ])
                  for n in ("wq", "wu", "wo"):
                      for c in range(DC):
                          nc.sync.dma_start(w_sb[n][:, c, :],
                                            w_ext[n][c * P:(c + 1) * P, :])

                  kv_ps = accps.tile([P, DC * P], f32, name="kv_ps")
                  for rt in range(RT):
                      pk0 = wps.tile([P, WIN], f32, name="pk0", tag="work")
                      pk1 = wps.tile([P, WIN], f32, name="pk1", tag="work")
                      pv0 = wps.tile([P, WIN], f32, name="pv0", tag="work")
                      pv1 = wps.tile([P, WIN], f32, name="pv1", tag="work")
                      for c in range(DC):
                          st, sto = c == 0, c == DC - 1
                          lhs = xT[c][:, rt * P:(rt + 1) * P]
                          nc.tensor.matmul(pk0[:], lhs, w_sb["wk"][:, c, 0:WIN],
                                           start=st, stop=sto)
                          nc.tensor.matmul(pk1[:], lhs, w_sb["wk"][:, c, WIN:D],
                                           start=st, stop=sto)
                      for c in range(DC):
                          st, sto = c == 0, c == DC - 1
                          lhs = xT[c][:, rt * P:(rt + 1) * P]
                          nc.tensor.matmul(pv0[:], lhs, w_sb["wv"][:, c, 0:WIN],
                                           start=st, stop=sto)
                          nc.tensor.matmul(pv1[:], lhs, w_sb["wv"][:, c, WIN:D],
                                           start=st, stop=sto)
                      k_bf = ab.tile([P, D], bf16, name="k_bf", tag="kvt", bufs=4)
                      v_bf = ab.tile([P, D], bf16, name="v_bf", tag="kvt", bufs=4)
                      kt = ab.tile([P, D], f32, name="kt", tag="ktmp", bufs=2)
                      nc.vector.tensor_tensor(kt[:, 0:WIN], pk0[:],
                                              bk_bc[:, 0:WIN], ALU.add)
                      nc.vector.tensor_tensor(kt[:, WIN:D], pk1[:],
                                              bk_bc[:, WIN:D], ALU.add)
                      nc.scalar.activation(k_bf[:, 0:WIN], kt[:, 0:WIN], AF.Relu)
                      nc.scalar.activation(k_bf[:, WIN:D], kt[:, WIN:D], AF.Relu)
                      nc.vector.tensor_tensor(v_bf[:, 0:WIN], pv0[:],
                                              bv_bc[:, 0:WIN], ALU.add)
                      nc.vector.tensor_tensor(v_bf[:, WIN:D], pv1[:],
                                              bv_bc[:, WIN:D], ALU.add)
                      for g in range(DC):
                          nc.tensor.matmul(
                              kv_ps[:, g * P:(g + 1) * P],
                              k_bf[:, g * P:(g + 1) * P],
                              v_bf[:, g * P:(g + 1) * P],
                              start=(rt == 0 and g % 4 == 0),
                              stop=(rt == RT - 1 and g % 4 == 3),
                          )

                  nc.vector.tensor_copy(kv_sb[:], kv_ps[:])

              # collective + kv prep (small outer-pool buffers only)
              if collective:
                  nc.sync.dma_start(kv_in[:], kv_sb[:])
                  nc.gpsimd.collective_compute(
                      "AllReduce", ALU.add, replica_groups=GROUPS,
                      ins=[kv_in[:]], outs=[kv_out[:]],
                  )
                  # readback overwrites kv_sb (partial no longer needed)
                  nc.sync.dma_start(kv_sb[:], kv_out[:])

              # clamp to [-100,100], |.| >= 0.01 keeping sign -- pipelined
              # per half/per g so attention g=0 unblocks early
              nc.vector.memset(kv_blk[:], 0.0)
              for h in range(2):
                  hs = slice(h * (DC * P // 2), (h + 1) * (DC * P // 2))
                  nc.vector.tensor_scalar(kv_sb[:, hs], kv_sb[:, hs],
                                          -100.0, 100.0,
                                          op0=ALU.max, op1=ALU.min)
                  nc.scalar.activation(kv_cl[:, hs], kv_sb[:, hs], AF.Sign)
                  nc.scalar.activation(kv_sb[:, hs], kv_sb[:, hs], AF.Abs)
                  nc.vector.tensor_scalar(kv_sb[:, hs], kv_sb[:, hs],
                                          0.01, None, op0=ALU.max)
                  for g in range(h * (DC // 2), (h + 1) * (DC // 2)):
                      gs = slice(g * P, (g + 1) * P)
                      nc.vector.tensor_tensor(kv_cl[:, gs], kv_cl[:, gs],
                                              kv_sb[:, gs], ALU.mult)
                      nc.vector.tensor_copy(kv_blk[0:HD, g * P:g * P + HD],
                                            kv_cl[0:HD, g * P:g * P + HD])
                      nc.vector.tensor_copy(
                          kv_blk[HD:P, g * P + HD:(g + 1) * P],
                          kv_cl[HD:P, g * P + HD:(g + 1) * P])

              # ---- fused phase BC, software pipelined:
              #   iteration i: S(i) q/u-project | A(i-LEAD) attn+LN+z |
              #                O(i-LEAD-PIPE) out-project
              # attention matmuls are interleaved into the S (or tail O)
              # matmul stream so two psum banks suffice with no PE stall
              with (
                  tc.tile_pool(name="pc", bufs=2) as pc,
                  tc.tile_pool(name="spsp", bufs=3, space="PSUM") as spsp,
                  tc.tile_pool(name="opsp", bufs=3, space="PSUM") as opsp,
                  tc.tile_pool(name="app", bufs=2, space="PSUM") as app,
              ):
                  qw_tiles, uw_tiles, zw_tiles = {}, {}, {}

                  def emit_O(w, inj=()):
                      zw = zw_tiles.pop(w)
                      subchunk = 0
                      inj = list(inj)
                      for j in range(RPW):
                          o0 = opsp.tile([P, WIN], f32, name="o0", tag="ops")
                          o1 = opsp.tile([P, WIN], f32, name="o1", tag="ops")
                          osb = pc.tile([P, D], bf16, name="osb", tag="osb",
                                        bufs=1)
                          for half, ot in ((0, o0), (1, o1)):
                              lo = half * WIN
                              for c in range(DC):
                                  nc.tensor.matmul(
                                      ot[:], zw[:, c, j * P:(j + 1) * P],
                                      w_sb["wo"][:, c, lo:lo + WIN],
                                      start=(c == 0), stop=(c == DC - 1))
                              nc.vector.scalar_tensor_tensor(
                                  osb[:, lo:lo + WIN], ot[:], 1.0,
                                  bo_b[:, lo:lo + WIN], ALU.mult, ALU.add)
                              if subchunk < len(inj):
                                  inj[subchunk]()
                              subchunk += 1
                          rt = w * RPW + j
                          nc.sync.dma_start(out_ext[rt * P:(rt + 1) * P, :],
                                            osb[:])

                  for i in range(NW + LEAD + PIPE):
                    do_S = i < NW
                    do_A = LEAD <= i < NW + LEAD
                    do_O = i >= LEAD + PIPE
                    wa = i - LEAD
                    wo = i - LEAD - PIPE

                    # A-stage attention matmuls (one [P,WIN] psum per
                    # head-block, issued interleaved into the S or O stream)
                    amms = []
                    if do_A:
                      q_wa = qw_tiles.pop(wa)
                      attn = pc.tile([P, DC, BWIN], bf16, name="attn",
                                     tag="attn", bufs=1)
                      sqw = pc.tile([P, DC, BWIN], bf16, name="sqw",
                                    tag="sqw", bufs=1)

                      def mk_amm(g, q_wa=q_wa, attn=attn, sqw=sqw):
                          def issue():
                              aps = app.tile([P, BWIN], f32, name="aps",
                                             tag="apair")
                              nc.tensor.matmul(
                                  aps[:], kv_blk[:, g * P:(g + 1) * P],
                                  q_wa[:, g, :], start=True, stop=True)
                              nc.scalar.activation(sqw[:, g, :], aps[:],
                                                   AF.Square)
                              nc.vector.tensor_copy(attn[:, g, :], aps[:])
                          return issue

                      amms = [mk_amm(g) for g in range(DC)]

                    # S stage: q/u projections, with A matmuls injected
                    # every other t-chunk (kv_blk is ready except for wa=0,
                    # where they would stall PE on the collective)
                    if do_S:
                      x_lo, x_hi = i * BWIN, (i + 1) * BWIN
                      q_w = pc.tile([P, DC, BWIN], bf16, name="q_w", tag="qw",
                                    bufs=LEAD + 1)
                      u_w = pc.tile([P, DC, BWIN], bf16, name="u_w", tag="uw",
                                    bufs=LEAD + 1)
                      inject = {}
                      if do_A and wa >= 1:
                          inject = {2 * g + 1: g for g in range(DC)}
                      chunk = 0
                      for nm, bias, func, dst in (
                          ("wq", bq_fm, AF.Relu, q_w),
                          ("wu", bu_fm, AF.Identity, u_w),
                      ):
                          for t in range(DC):
                              ps = spsp.tile([P, BWIN], f32, name="pqu",
                                             tag="sps")
                              for c in range(DC):
                                  nc.tensor.matmul(
                                      ps[:], w_sb[nm][:, c, t * P:(t + 1) * P],
                                      xT[c][:, x_lo:x_hi],
                                      start=(c == 0), stop=(c == DC - 1))
                              nc.scalar.activation(dst[:, t, :], ps[:], func,
                                                   bias=bias[:, t:t + 1],
                                                   scale=1.0)
                              if chunk in inject:
                                  amms[inject[chunk]]()
                              chunk += 1
                      qw_tiles[i] = q_w
                      uw_tiles[i] = u_w
                      if do_A and wa < 1:
                          for p in amms:
                              p()
                    elif do_O:
                      # tail: out-projection first with the A matmuls
                      # injected into its stream; LN/z of window wa follows
                      emit_O(wo, amms)
                    elif do_A:
                      for p in amms:
                          p()

                    if do_A:
                      u_wa = uw_tiles[wa]
                      # LN stats: partial sums on the Pool engine + gpsimd
                      # partition all-reduce (DVE is reserved for the
                      # z-chain; PE stays busy with S/O matmuls)
                      s_b = pc.tile([P, BWIN], f32, name="s_b", tag="s_b",
                                    bufs=1)
                      q_b = pc.tile([P, BWIN], f32, name="q_b", tag="q_b",
                                    bufs=1)
                      s_pt = pc.tile([P, BWIN], f32, name="s_pt",
                                     tag="s_pt", bufs=1)
                      q_pt = pc.tile([P, BWIN], f32, name="q_pt",
                                     tag="q_pt", bufs=1)
                      nc.vector.tensor_tensor(s_pt[:], attn[:, 0, :],
                                              attn[:, 1, :], ALU.add)
                      nc.vector.tensor_tensor(q_pt[:], sqw[:, 0, :],
                                              sqw[:, 1, :], ALU.add)
                      for g in range(2, DC):
                          nc.vector.tensor_tensor(s_pt[:], s_pt[:],
                                                  attn[:, g, :], ALU.add)
                          nc.vector.tensor_tensor(q_pt[:], q_pt[:],
                                                  sqw[:, g, :], ALU.add)
                      nc.gpsimd.partition_all_reduce(
                          s_b[:], s_pt[:], P, bass_isa.ReduceOp.add)
                      nc.gpsimd.partition_all_reduce(
                          q_b[:], q_pt[:], P, bass_isa.ReduceOp.add)
                      # in-place LN scalars: s_b -> mean, q_b -> 1/(var+eps),
                      # s_pt reused as mean^2 scratch
                      nc.vector.tensor_scalar(s_b[:], s_b[:], 1.0 / D, None,
                                              op0=ALU.mult)
                      nc.vector.tensor_tensor(s_pt[:], s_b[:], s_b[:],
                                              ALU.mult)
                      nc.vector.scalar_tensor_tensor(q_b[:], q_b[:], 1.0 / D,
                                                     s_pt[:], ALU.mult,
                                                     ALU.subtract)
                      nc.vector.tensor_scalar(q_b[:], q_b[:], EPS, None,
                                              op0=ALU.add)
                      nc.vector.reciprocal(q_b[:], q_b[:])
                      rstd_b = pc.tile([P, BWIN], bf16, name="rstd_b",
                                       tag="rstd_b")
                      nc.scalar.activation(rstd_b[:], q_b[:], AF.Sqrt)

                      zw = pc.tile([P, DC, BWIN], bf16, name="zw", tag="zw",
                                   bufs=PIPE + 1)
                      if fold_ln:
                          # ln_w==1, ln_b==0:
                          # z = (attn - mu) * rstd * u  (3 DVE ops per g)
                          negmu = pc.tile([P, BWIN], bf16, name="negmu",
                                          tag="negmu", bufs=1)
                          nc.vector.tensor_scalar(negmu[:], s_b[:], -1.0,
                                                  None, op0=ALU.mult)
                          for g in range(DC):
                              pg = pc.tile([P, BWIN], bf16, name="pg",
                                           tag="pg", bufs=2)
                              nc.vector.tensor_tensor(pg[:], rstd_b[:],
                                                      u_wa[:, g, :], ALU.mult)
                              zt = pc.tile([P, BWIN], bf16, name="zt",
                                           tag="zt", bufs=2)
                              nc.vector.tensor_tensor(zt[:], attn[:, g, :],
                                                      negmu[:], ALU.add)
                              nc.vector.tensor_tensor(zw[:, g, :], zt[:],
                                                      pg[:], ALU.mult)
                      else:
                          # z = ((attn*rstd - mu*rstd) * lnw + lnb) * u
                          shp_b = pc.tile([P, BWIN], bf16, name="shp_b",
                                          tag="shp_b")
                          nc.vector.scalar_tensor_tensor(shp_b[:], s_b[:],
                                                         -1.0, rstd_b[:],
                                                         ALU.mult, ALU.mult)
                          for g in range(DC):
                              zt = pc.tile([P, BWIN], bf16, name="zt",
                                           tag="zt", bufs=2)
                              nc.vector.tensor_tensor(zt[:], attn[:, g, :],
                                                      rstd_b[:], ALU.mult)
                              nc.vector.tensor_tensor(zt[:], zt[:], shp_b[:],
                                                      ALU.add)
                              nc.vector.tensor_scalar(zt[:], zt[:],
                                                      lnw_fm[:, g:g + 1],
                                                      lnb_fm[:, g:g + 1],
                                                      op0=ALU.mult,
                                                      op1=ALU.add)
                              nc.vector.tensor_tensor(zw[:, g, :], zt[:],
                                                      u_wa[:, g, :], ALU.mult)
                      del uw_tiles[wa]
                      zw_tiles[wa] = zw

                    if do_O and do_S:
                      emit_O(wo)

    nc.compile()
    return nc


def make_in_maps(query, Wq, bq, Wk, bk, Wv, bv, Wu, bu, Wo, bo, ln_w, ln_b,
                 R=R_FULL):
    xs = query.reshape(-1, D).astype(NPBF16)
    common = {
        "wk": np.ascontiguousarray(Wk).astype(NPBF16),
        "wv": np.ascontiguousarray(Wv).astype(NPBF16),
        "wq": np.ascontiguousarray(Wq).astype(NPBF16),
        "wu": np.ascontiguousarray(Wu).astype(NPBF16),
        "wo": np.ascontiguousarray(Wo).astype(NPBF16),
        "bk_row": np.ascontiguousarray(bk.astype(NPBF16).reshape(1, D)),
        "bv_row": np.ascontiguousarray(bv.astype(NPBF16).reshape(1, D)),
        "bo_b": np.ascontiguousarray(
            np.broadcast_to(bo.astype(NPBF16), (P, D))),
        "bq_fm": np.ascontiguousarray(bq.astype(np.float32).reshape(DC, P).T),
        "bu_fm": np.ascontiguousarray(bu.astype(np.float32).reshape(DC, P).T),
        "lnw_fm": np.ascontiguousarray(ln_w.astype(np.float32).reshape(DC, P).T),
        "lnb_fm": np.ascontiguousarray(ln_b.astype(np.float32).reshape(DC, P).T),
    }
    return [dict(common, xt=np.ascontiguousarray(
                xs[c * R:(c + 1) * R].T.reshape(DC, P, R)))
            for c in range(NCORES)]


_NC_CACHE = {}


def kernel(query, Wq, bq, Wk, bk, Wv, bv, Wu, bu, Wo, bo, ln_w, ln_b):
    query = np.asarray(query, dtype=np.float32)
    fold = bool(np.all(np.asarray(ln_w) == 1.0)
                and np.all(np.asarray(ln_b) == 0.0))
    key = ("nc", fold)
    if key not in _NC_CACHE:
        _NC_CACHE[key] = build(fold_ln=fold)
    nc = _NC_CACHE[key]
    in_maps = make_in_maps(query, np.asarray(Wq), np.asarray(bq),
                           np.asarray(Wk), np.asarray(bk),
                           np.asarray(Wv), np.asarray(bv),
                           np.asarray(Wu), np.asarray(bu),
                           np.asarray(Wo), np.asarray(bo),
                           np.asarray(ln_w), np.asarray(ln_b))
    res = run_bass_kernel_spmd(nc, in_maps, list(range(NCORES)))
    out = np.empty((B * N, D), np.float32)
    for c in range(NCORES):
        out[c * R_FULL:(c + 1) * R_FULL] = res.results[c]["out"].astype(
            np.float32)
    return out.reshape(B, N, D)
